# revision 15
# baseline (speedup 1.0000x reference)
"""Causal self-attention kernel for Trainium2 (8 NeuronCores, data-parallel).

Shapes (hardcoded): x [8, 1024, 640], qkv_w [1920, 640], qkv_b [1920],
out_w [640, 640], out_b [640].  B=8 batch elements -> one per core.

Per-core layout strategy: keep activations feature-major ("transposed") so
every matmul contraction dim sits on SBUF partitions:
  xT [640, 1024]          (host-transposed input)
  qT = q_wT.T @ xT        [640, 1024] chunks (+bias)
  kT -> k_pad_e/k_pad_o   zero-padded to K=128 so QK matmuls use the full
                          PE array (half-array matmuls don't register as
                          activity for the HAM clock gate -> 1.2GHz)
  v   = xT.T @ v_wT       [1024, 640]   (k-position major, ready as AV lhsT)
  attT[k,q] = kT.T @ qT   per head, exp with causal mask, no max-subtract
                          (|logits| <= ~1.4 for this problem's scale)
  yT_aug = v_aug.T @ expT [65+, q]      (row 64 = softmax denominators)
  outT = out_wT.T @ yT    [640, 1024]   (host transposes back)
All matmuls run as float32r (1 cycle/row for N>=256 vs 4 for fp32).
v-projection bias is folded into the output bias on the host (softmax rows
sum to 1, so att @ (1 v_b^T) == 1 v_b^T exactly).
Phase-B leftovers and the first half of the output projection are emitted
as fillers inside the attention loop to keep the PE fed while ACT/DVE work
on exp/normalization.
"""

import numpy as np

B, L, D = 8, 1024, 640
H, DH = 10, 64
DC = D // 128          # 5 contraction chunks
SCALE = DH ** -0.5

_cache = {}


def _build():
    import contextlib

    import concourse.bacc as bacc
    import concourse.mybir as mybir
    import concourse.tile as tile

    f32 = mybir.dt.float32
    f32r = mybir.dt.float32r
    Act = mybir.ActivationFunctionType

    nc = bacc.Bacc("TRN2", target_bir_lowering=False)

    xt_d = nc.dram_tensor("xt", [D, L], f32r, kind="ExternalInput")
    qkw_d = nc.dram_tensor("qkw", [D, 2 * D], f32r, kind="ExternalInput")
    vw_d = nc.dram_tensor("vw", [D, D], f32r, kind="ExternalInput")
    ow_d = nc.dram_tensor("ow", [D, D], f32r, kind="ExternalInput")
    qkb_d = nc.dram_tensor("qkb", [128, 2 * DC], f32, kind="ExternalInput")
    outb_d = nc.dram_tensor("outb", [128, DC], f32, kind="ExternalInput")
    mask_d = nc.dram_tensor("mask", [128, 128], f32r, kind="ExternalInput")
    ones_d = nc.dram_tensor("ones", [128, 1], f32r, kind="ExternalInput")
    zeros_d = nc.dram_tensor("zeros", [128, L], f32r, kind="ExternalInput")
    o_d = nc.dram_tensor("o", [D, L], f32, kind="ExternalOutput")

    with tile.TileContext(nc) as tc:
        with contextlib.ExitStack() as ctx:
            singles = ctx.enter_context(tc.tile_pool(name="singles", bufs=1))
            expp = ctx.enter_context(tc.tile_pool(name="expp", bufs=4))
            normp = ctx.enter_context(tc.tile_pool(name="normp", bufs=2))
            outp = ctx.enter_context(tc.tile_pool(name="outp", bufs=3))
            owp = ctx.enter_context(tc.tile_pool(name="owp", bufs=6))
            ps_proj = ctx.enter_context(tc.tile_pool(name="ps_proj", bufs=2, space="PSUM"))
            ps_att = ctx.enter_context(tc.tile_pool(name="ps_att", bufs=3, space="PSUM"))
            ps_y = ctx.enter_context(tc.tile_pool(name="ps_y", bufs=3, space="PSUM"))

            # ---- resident SBUF tensors ----
            xt_s = singles.tile([128, DC, L], f32r)        # xT
            qkw_s = singles.tile([128, DC, 2 * D], f32r)   # [q_wT | k_wT]
            vw_s = singles.tile([128, DC, D], f32r)        # v_wT
            q_s = singles.tile([128, DC, L], f32r)         # qT (+bias)
            k_pad_e = singles.tile([128, DC, L], f32r)     # [kT_even; 0]
            k_pad_o = singles.tile([128, DC, L], f32r)     # [0; kT_odd]
            v_s = singles.tile([128, L // 128, H * 65 + 63], f32r)
            yt_s = singles.tile([128, DC, L], f32r)        # normalized yT
            qkb_s = singles.tile([128, 2 * DC], f32)
            outb_s = singles.tile([128, DC], f32)
            mask_s = singles.tile([128, 128], f32r)

            # ---- input DMAs (emission order ~= queue order) ----
            for dc in range(DC):
                nc.sync.dma_start(out=xt_s[:, dc, :], in_=xt_d[128 * dc:128 * dc + 128, :])
            nc.sync.dma_start(out=qkb_s, in_=qkb_d[:, :])
            # qkw per (dc, mi) slice so early B groups aren't gated on the full load.
            # mi order: q0,k0,q1,k1,... (pair-critical first)
            mi_order = [x for p in range(DC) for x in (p, DC + p)]
            for mi in mi_order:
                for dc in range(DC):
                    nc.sync.dma_start(
                        out=qkw_s[:, dc, 128 * mi:128 * mi + 128],
                        in_=qkw_d[128 * dc:128 * dc + 128, 128 * mi:128 * mi + 128])
            for dc in range(DC):
                nc.sync.dma_start(out=vw_s[:, dc, :], in_=vw_d[128 * dc:128 * dc + 128, :])
            nc.sync.dma_start(out=mask_s, in_=mask_d[:, :])
            nc.sync.dma_start(out=outb_s, in_=outb_d[:, :])
            # constant halves of the zero-padded kT tensors
            for dc in range(DC):
                nc.sync.dma_start(out=k_pad_e[64:128, dc, :], in_=zeros_d[0:64, :])
                nc.sync.dma_start(out=k_pad_o[0:64, dc, :], in_=zeros_d[0:64, :])
            # ones columns of v_aug + zero tail padding
            for li in range(L // 128):
                dst = v_s[:, li, 0:650].rearrange("p (h c) -> p h c", c=65)[:, :, 64:65]
                nc.sync.dma_start(out=dst, in_=ones_d[:, :].to_broadcast((128, H, 1)))
                nc.sync.dma_start(out=v_s[:, li, H * 65:], in_=zeros_d[:, 0:63])

            # ---- phase B pieces: one (mi, nn) PSUM group = 5 matmuls + ACT ----
            def b_group_steps(mi, nn):
                hold = {}

                def pt():
                    if "pt" not in hold:
                        hold["pt"] = ps_proj.tile([128, 512], f32, tag="proj", name="ptl")
                    return hold["pt"]
                mms = []
                for dc in range(DC):
                    mms.append(lambda dc=dc: nc.tensor.matmul(
                        pt()[:, :],
                        qkw_s[:, dc, 128 * mi:128 * mi + 128],
                        xt_s[:, dc, 512 * nn:512 * nn + 512],
                        start=(dc == 0), stop=(dc == DC - 1)))

                def finish(mi=mi, nn=nn):
                    pt = hold["pt"]
                    sl = slice(512 * nn, 512 * nn + 512)
                    if mi < DC:            # q chunk
                        nc.scalar.activation(q_s[:, mi, sl], pt[:, :],
                                             Act.Identity, bias=qkb_s[:, mi:mi + 1])
                    else:                  # k chunk -> zero-padded halves
                        hp = mi - DC
                        nc.scalar.activation(k_pad_e[0:64, hp, sl], pt[0:64, :],
                                             Act.Identity, bias=qkb_s[0:64, mi:mi + 1])
                        nc.scalar.activation(k_pad_o[64:128, hp, sl], pt[64:128, :],
                                             Act.Identity, bias=qkb_s[64:128, mi:mi + 1])
                return mms + [finish]

            def emit_b_chunk(mi):
                for nn in range(2):
                    for step in b_group_steps(mi, nn):
                        step()

            # ---- phase E pieces: one (jc, nn) group ----
            def e_group_steps(jc, nn):
                ow_tiles = []
                hold = {}

                def pt():
                    if "pt" not in hold:
                        hold["pt"] = ps_proj.tile([128, 512], f32, tag="proj", name="ptl")
                    return hold["pt"]

                def load(jc=jc):
                    for dc in range(DC):
                        t = owp.tile([128, 128], f32r, tag="ow", name="owt")
                        nc.sync.dma_start(
                            out=t,
                            in_=ow_d[128 * dc:128 * dc + 128, 128 * jc:128 * jc + 128])
                        ow_tiles.append(t)
                mms = []
                for dc in range(DC):
                    mms.append(lambda dc=dc: nc.tensor.matmul(
                        pt()[:, :],
                        ow_tiles[dc][:, :],
                        yt_s[:, dc, 512 * nn:512 * nn + 512],
                        start=(dc == 0), stop=(dc == DC - 1)))

                def finish(jc=jc, nn=nn):
                    ot = outp.tile([128, 512], f32, tag="ot")
                    nc.scalar.activation(ot[:, :], hold["pt"][:, :],
                                         Act.Identity, bias=outb_s[:, jc:jc + 1])
                    nc.sync.dma_start(
                        out=o_d[128 * jc:128 * jc + 128, 512 * nn:512 * nn + 512],
                        in_=ot[:, :])
                return [load] + mms + [finish]

            # ---- phase C: v = xT.T @ v_wT ----
            def emit_c():
                for li in range(L // 128):
                    pt = ps_proj.tile([128, 512], f32, tag="proj")
                    for dc in range(DC):
                        nc.tensor.matmul(
                            pt[:, :],
                            xt_s[:, dc, 128 * li:128 * li + 128],
                            vw_s[:, dc, 0:512],
                            start=(dc == 0), stop=(dc == DC - 1))
                    dst = v_s[:, li, 0:520].rearrange("p (h c) -> p h c", c=65)[:, :, 0:64]
                    nc.vector.tensor_copy(out=dst, in_=pt[:, :].rearrange("p (h c) -> p h c", c=64))
                    pt2 = ps_proj.tile([128, 128], f32, tag="proj")
                    for dc in range(DC):
                        nc.tensor.matmul(
                            pt2[:, :],
                            xt_s[:, dc, 128 * li:128 * li + 128],
                            vw_s[:, dc, 512:640],
                            start=(dc == 0), stop=(dc == DC - 1))
                    dst2 = v_s[:, li, 520:650].rearrange("p (h c) -> p h c", c=65)[:, :, 0:64]
                    nc.vector.tensor_copy(out=dst2, in_=pt2[:, :].rearrange("p (h c) -> p h c", c=64))

            # ---- phase D: attention; `fillers` feed the PE during stalls ----
            fillers = []

            def pop_filler():
                if fillers:
                    fillers.pop(0)()

            def flush_fillers():
                while fillers:
                    fillers.pop(0)()

            def attend(h, qc):
                kp = k_pad_o if (h % 2) else k_pad_e
                po = 64 * (h % 2)
                qT2 = q_s[:, h // 2, :]
                py = ps_y.tile([128, 512], f32, tag="py")
                nki = 4 * qc + 4
                for ki in range(nki):
                    off = 128 * ki - 512 * qc
                    lo = max(0, off)
                    pa = ps_att.tile([128, 512], f32, tag="pa")
                    nc.tensor.matmul(
                        pa[:, lo:512],
                        kp[:, h // 2, 128 * ki:128 * ki + 128],
                        qT2[:, 512 * qc + lo:512 * qc + 512],
                        start=True, stop=True)
                    ex = expp.tile([128, 512], f32r, tag="ex")
                    nc.scalar.activation(ex[:, lo:512], pa[:, lo:512], Act.Exp, scale=SCALE)
                    if off >= 0:
                        eng = nc.vector if (ki % 2 == 0) else nc.gpsimd
                        eng.tensor_mul(
                            ex[:, off:off + 128], ex[:, off:off + 128], mask_s[:, :])
                    nc.tensor.matmul(
                        py[:, lo:512],
                        v_s[:, ki, 65 * h:65 * h + 128],
                        ex[:, lo:512],
                        start=(ki == 0), stop=(ki == nki - 1))
                    pop_filler()
                den = normp.tile([1, 512], f32, tag="den")
                nc.vector.tensor_copy(out=den[:, :], in_=py[64:65, :])
                recip = normp.tile([1, 512], f32, tag="rc")
                nc.vector.reciprocal_approx_fast(out=recip[:, :], in_=den[:, :])
                rb = normp.tile([64, 512], f32, tag="rb")
                nc.gpsimd.partition_broadcast(rb[:, :], recip[:, :])
                nc.vector.tensor_mul(
                    yt_s[po:po + 64, h // 2, 512 * qc:512 * qc + 512],
                    py[0:64, :], rb[:, :])

            # ---- emission schedule ----
            emit_b_chunk(0)
            emit_b_chunk(DC + 0)
            emit_b_chunk(1)
            emit_b_chunk(DC + 1)
            emit_c()

            for qc in range(2):
                for p in range(DC):
                    if qc == 0 and p >= 2:
                        flush_fillers()   # pair p needs B chunks staged at p-2
                    if qc == 0 and p + 2 < DC:
                        # stage pair p+2's B chunks as fillers
                        for mi in (p + 2, DC + p + 2):
                            for nn in range(2):
                                fillers.extend(b_group_steps(mi, nn))
                    if qc == 1:
                        # stage E(nn=0) groups as fillers (yt cols 0:512 are done)
                        fillers.extend(e_group_steps(p, 0))
                    attend(2 * p, qc)
                    attend(2 * p + 1, qc)
            flush_fillers()

            # ---- phase E second half ----
            for jc in range(DC):
                for step in e_group_steps(jc, 1):
                    step()

    nc.compile()
    return nc


def _prep_inputs(x, qkv_w, qkv_b, out_w, out_b):
    x = np.ascontiguousarray(x, dtype=np.float32)
    qkv_w = np.asarray(qkv_w, dtype=np.float32)
    qkv_b = np.asarray(qkv_b, dtype=np.float32)
    out_w = np.asarray(out_w, dtype=np.float32)
    out_b = np.asarray(out_b, dtype=np.float32)

    xT = np.ascontiguousarray(x.transpose(0, 2, 1))              # [B, D, L]
    qk_wT = np.ascontiguousarray(qkv_w[:2 * D].T)                # [D, 2D]
    v_wT = np.ascontiguousarray(qkv_w[2 * D:].T)                 # [D, D]
    out_wT = np.ascontiguousarray(out_w.T)                       # [D, D]
    qkb = np.ascontiguousarray(qkv_b[:2 * D].reshape(2 * DC, 128).T)
    out_b_eff = out_b + out_w @ qkv_b[2 * D:]
    outb = np.ascontiguousarray(out_b_eff.reshape(DC, 128).T)
    kk, qq = np.meshgrid(np.arange(128), np.arange(128), indexing="ij")
    mask = (kk <= qq).astype(np.float32)
    ones = np.ones((128, 1), np.float32)
    zeros = np.zeros((128, L), np.float32)

    shared = {"qkw": qk_wT, "vw": v_wT, "ow": out_wT, "qkb": qkb,
              "outb": outb, "mask": mask, "ones": ones, "zeros": zeros}
    return [dict(shared, xt=xT[c]) for c in range(B)]


def kernel(x, qkv_w, qkv_b, out_w, out_b):
    from concourse.bass_utils import run_bass_kernel_spmd

    if "nc" not in _cache:
        _cache["nc"] = _build()
    nc = _cache["nc"]

    in_maps = _prep_inputs(x, qkv_w, qkv_b, out_w, out_b)
    res = run_bass_kernel_spmd(nc, in_maps, core_ids=list(range(B)))
    out = np.empty((B, L, D), np.float32)
    for c in range(B):
        out[c] = res.results[c]["o"].T
    return out


# revision 16
# speedup vs baseline: 1.0575x; 1.0575x over previous
"""Causal self-attention kernel for Trainium2 (8 NeuronCores, data-parallel).

Shapes (hardcoded): x [8, 1024, 640], qkv_w [1920, 640], qkv_b [1920],
out_w [640, 640], out_b [640].  B=8 batch elements -> one per core.

Per-core layout strategy: keep activations feature-major ("transposed") so
every matmul contraction dim sits on SBUF partitions:
  xT [640, 1024]          (host-transposed input)
  qT = q_wT.T @ xT        [640, 1024] chunks (+bias)
  kT -> k_pad_e/k_pad_o   zero-padded to K=128 so QK matmuls use the full
                          PE array (half-array matmuls don't register as
                          activity for the HAM clock gate -> 1.2GHz)
  v   = xT.T @ v_wT       [1024, 640]   (k-position major, ready as AV lhsT)
  attT[k,q] = kT.T @ qT   per head, exp with causal mask, no max-subtract
                          (|logits| <= ~1.4 for this problem's scale)
  yT_aug = v_aug.T @ expT [65+, q]      (row 64 = softmax denominators)
  outT = out_wT.T @ yT    [640, 1024]   (host transposes back)
All matmuls run as float32r (1 cycle/row for N>=256 vs 4 for fp32).
v-projection bias is folded into the output bias on the host (softmax rows
sum to 1, so att @ (1 v_b^T) == 1 v_b^T exactly).
Phase-B leftovers and the first half of the output projection are emitted
as fillers inside the attention loop to keep the PE fed while ACT/DVE work
on exp/normalization.
"""

import numpy as np

B, L, D = 8, 1024, 640
H, DH = 10, 64
DC = D // 128          # 5 contraction chunks
SCALE = DH ** -0.5

_cache = {}


def _build():
    import contextlib

    import concourse.bacc as bacc
    import concourse.mybir as mybir
    import concourse.tile as tile

    f32 = mybir.dt.float32
    f32r = mybir.dt.float32r
    Act = mybir.ActivationFunctionType

    nc = bacc.Bacc("TRN2", target_bir_lowering=False)

    xt_d = nc.dram_tensor("xt", [D, L], f32r, kind="ExternalInput")
    qkw_d = nc.dram_tensor("qkw", [D, 2 * D], f32r, kind="ExternalInput")
    vw_d = nc.dram_tensor("vw", [D, D], f32r, kind="ExternalInput")
    ow_d = nc.dram_tensor("ow", [D, D], f32r, kind="ExternalInput")
    qkb_d = nc.dram_tensor("qkb", [128, 2 * DC], f32, kind="ExternalInput")
    outb_d = nc.dram_tensor("outb", [128, DC], f32, kind="ExternalInput")
    mask_d = nc.dram_tensor("mask", [128, 128], f32r, kind="ExternalInput")
    ones_d = nc.dram_tensor("ones", [128, 1], f32r, kind="ExternalInput")
    zeros_d = nc.dram_tensor("zeros", [128, L], f32r, kind="ExternalInput")
    o_d = nc.dram_tensor("o", [D, L], f32, kind="ExternalOutput")

    with tile.TileContext(nc) as tc:
        with contextlib.ExitStack() as ctx:
            singles = ctx.enter_context(tc.tile_pool(name="singles", bufs=1))
            expp = ctx.enter_context(tc.tile_pool(name="expp", bufs=4))
            normp = ctx.enter_context(tc.tile_pool(name="normp", bufs=4))
            outp = ctx.enter_context(tc.tile_pool(name="outp", bufs=3))
            owp = ctx.enter_context(tc.tile_pool(name="owp", bufs=6))
            ps_proj = ctx.enter_context(tc.tile_pool(name="ps_proj", bufs=2, space="PSUM"))
            ps_att = ctx.enter_context(tc.tile_pool(name="ps_att", bufs=3, space="PSUM"))
            ps_y = ctx.enter_context(tc.tile_pool(name="ps_y", bufs=3, space="PSUM"))

            # ---- resident SBUF tensors ----
            xt_s = singles.tile([128, DC, L], f32r)        # xT
            qkw_s = singles.tile([128, DC, 2 * D], f32r)   # [q_wT | k_wT]
            vw_s = singles.tile([128, DC, D], f32r)        # v_wT
            q_s = singles.tile([128, DC, L], f32r)         # qT (+bias)
            k_pad_e = singles.tile([128, DC, L], f32r)     # [kT_even; 0]
            k_pad_o = singles.tile([128, DC, L], f32r)     # [0; kT_odd]
            v_s = singles.tile([128, L // 128, H * 65 + 63], f32r)
            yt_s = singles.tile([128, DC, L], f32r)        # normalized yT
            qkb_s = singles.tile([128, 2 * DC], f32)
            outb_s = singles.tile([128, DC], f32)
            mask_s = singles.tile([128, 128], f32r)

            # ---- input DMAs (emission order ~= queue order) ----
            for dc in range(DC):
                nc.sync.dma_start(out=xt_s[:, dc, :], in_=xt_d[128 * dc:128 * dc + 128, :])
            nc.sync.dma_start(out=qkb_s, in_=qkb_d[:, :])
            # q-half of qkw first (unblocks B), then vw (unblocks C), then k-half
            for dc in range(DC):
                nc.sync.dma_start(out=qkw_s[:, dc, 0:D],
                                  in_=qkw_d[128 * dc:128 * dc + 128, 0:D])
            for dc in range(DC):
                nc.sync.dma_start(out=vw_s[:, dc, :], in_=vw_d[128 * dc:128 * dc + 128, :])
            for dc in range(DC):
                nc.sync.dma_start(out=qkw_s[:, dc, D:2 * D],
                                  in_=qkw_d[128 * dc:128 * dc + 128, D:2 * D])
            nc.sync.dma_start(out=mask_s, in_=mask_d[:, :])
            nc.sync.dma_start(out=outb_s, in_=outb_d[:, :])
            # constant halves of the zero-padded kT tensors
            for dc in range(DC):
                nc.sync.dma_start(out=k_pad_e[64:128, dc, :], in_=zeros_d[0:64, :])
                nc.sync.dma_start(out=k_pad_o[0:64, dc, :], in_=zeros_d[0:64, :])
            # ones columns of v_aug + zero tail padding
            for li in range(L // 128):
                dst = v_s[:, li, 0:650].rearrange("p (h c) -> p h c", c=65)[:, :, 64:65]
                nc.sync.dma_start(out=dst, in_=ones_d[:, :].to_broadcast((128, H, 1)))
                nc.sync.dma_start(out=v_s[:, li, H * 65:], in_=zeros_d[:, 0:63])

            # ---- phase B: one (mi, nn) PSUM group = 5 matmuls + ACT ----
            def emit_b_group(mi, nn):
                pt = ps_proj.tile([128, 512], f32, tag="proj", name="ptl")
                for dc in range(DC):
                    nc.tensor.matmul(
                        pt[:, :],
                        qkw_s[:, dc, 128 * mi:128 * mi + 128],
                        xt_s[:, dc, 512 * nn:512 * nn + 512],
                        start=(dc == 0), stop=(dc == DC - 1))
                sl = slice(512 * nn, 512 * nn + 512)
                if mi < DC:            # q chunk
                    nc.scalar.activation(q_s[:, mi, sl], pt[:, :],
                                         Act.Identity, bias=qkb_s[:, mi:mi + 1])
                else:                  # k chunk -> zero-padded halves
                    hp = mi - DC
                    nc.scalar.activation(k_pad_e[0:64, hp, sl], pt[0:64, :],
                                         Act.Identity, bias=qkb_s[0:64, mi:mi + 1])
                    nc.scalar.activation(k_pad_o[64:128, hp, sl], pt[64:128, :],
                                         Act.Identity, bias=qkb_s[64:128, mi:mi + 1])

            def emit_b_chunk(mi):
                for nn in range(2):
                    emit_b_group(mi, nn)

            # ---- phase E: one jc group (ow streamed from DRAM) ----
            def emit_e_chunk(jc):
                ow_tiles = []
                for dc in range(DC):
                    t = owp.tile([128, 128], f32r, tag="ow", name="owt")
                    nc.sync.dma_start(
                        out=t,
                        in_=ow_d[128 * dc:128 * dc + 128, 128 * jc:128 * jc + 128])
                    ow_tiles.append(t)
                for nn in range(2):
                    pt = ps_proj.tile([128, 512], f32, tag="proj", name="ptl")
                    for dc in range(DC):
                        nc.tensor.matmul(
                            pt[:, :],
                            ow_tiles[dc][:, :],
                            yt_s[:, dc, 512 * nn:512 * nn + 512],
                            start=(dc == 0), stop=(dc == DC - 1))
                    ot = outp.tile([128, 512], f32, tag="ot")
                    nc.scalar.activation(ot[:, :], pt[:, :],
                                         Act.Identity, bias=outb_s[:, jc:jc + 1])
                    nc.sync.dma_start(
                        out=o_d[128 * jc:128 * jc + 128, 512 * nn:512 * nn + 512],
                        in_=ot[:, :])

            # ---- phase C: v = xT.T @ v_wT ----
            def emit_c():
                for li in range(L // 128):
                    pt = ps_proj.tile([128, 512], f32, tag="proj")
                    for dc in range(DC):
                        nc.tensor.matmul(
                            pt[:, :],
                            xt_s[:, dc, 128 * li:128 * li + 128],
                            vw_s[:, dc, 0:512],
                            start=(dc == 0), stop=(dc == DC - 1))
                    dst = v_s[:, li, 0:520].rearrange("p (h c) -> p h c", c=65)[:, :, 0:64]
                    nc.vector.tensor_copy(out=dst, in_=pt[:, :].rearrange("p (h c) -> p h c", c=64))
                    pt2 = ps_proj.tile([128, 128], f32, tag="proj")
                    for dc in range(DC):
                        nc.tensor.matmul(
                            pt2[:, :],
                            xt_s[:, dc, 128 * li:128 * li + 128],
                            vw_s[:, dc, 512:640],
                            start=(dc == 0), stop=(dc == DC - 1))
                    dst2 = v_s[:, li, 520:650].rearrange("p (h c) -> p h c", c=65)[:, :, 0:64]
                    nc.vector.tensor_copy(out=dst2, in_=pt2[:, :].rearrange("p (h c) -> p h c", c=64))

            # ---- phase D: attention ----
            def attend(h, qc):
                kp = k_pad_o if (h % 2) else k_pad_e
                po = 64 * (h % 2)
                qT2 = q_s[:, h // 2, :]
                py = ps_y.tile([128, 512], f32, tag="py")
                nki = 4 * qc + 4
                for ki in range(nki):
                    off = 128 * ki - 512 * qc
                    lo = max(0, off)
                    pa = ps_att.tile([128, 512], f32, tag="pa")
                    nc.tensor.matmul(
                        pa[:, lo:512],
                        kp[:, h // 2, 128 * ki:128 * ki + 128],
                        qT2[:, 512 * qc + lo:512 * qc + 512],
                        start=True, stop=True)
                    ex = expp.tile([128, 512], f32r, tag="ex")
                    nc.scalar.activation(ex[:, lo:512], pa[:, lo:512], Act.Exp, scale=SCALE)
                    if off >= 0:
                        eng = nc.vector if (ki % 2 == 0) else nc.gpsimd
                        eng.tensor_mul(
                            ex[:, off:off + 128], ex[:, off:off + 128], mask_s[:, :])
                    nc.tensor.matmul(
                        py[:, lo:512],
                        v_s[:, ki, 65 * h:65 * h + 128],
                        ex[:, lo:512],
                        start=(ki == 0), stop=(ki == nki - 1))
                den = normp.tile([1, 512], f32, tag="den")
                nc.vector.tensor_copy(out=den[:, :], in_=py[64:65, :])
                recip = normp.tile([1, 512], f32, tag="rc")
                nc.vector.reciprocal_approx_fast(out=recip[:, :], in_=den[:, :])
                rb = normp.tile([64, 512], f32, tag="rb")
                nc.gpsimd.partition_broadcast(rb[:, :], recip[:, :])
                nc.vector.tensor_mul(
                    yt_s[po:po + 64, h // 2, 512 * qc:512 * qc + 512],
                    py[0:64, :], rb[:, :])

            # ---- emission schedule ----
            for mi in range(DC):
                emit_b_chunk(mi)          # q chunks
            for mi in range(DC, 2 * DC):
                emit_b_chunk(mi)          # k chunks -> k_pad
            emit_c()
            for p in range(DC):
                for qc in range(2):
                    attend(2 * p, qc)
                    attend(2 * p + 1, qc)
            for jc in range(DC):
                emit_e_chunk(jc)

    nc.compile()
    return nc


def _prep_inputs(x, qkv_w, qkv_b, out_w, out_b):
    x = np.ascontiguousarray(x, dtype=np.float32)
    qkv_w = np.asarray(qkv_w, dtype=np.float32)
    qkv_b = np.asarray(qkv_b, dtype=np.float32)
    out_w = np.asarray(out_w, dtype=np.float32)
    out_b = np.asarray(out_b, dtype=np.float32)

    xT = np.ascontiguousarray(x.transpose(0, 2, 1))              # [B, D, L]
    qk_wT = np.ascontiguousarray(qkv_w[:2 * D].T)                # [D, 2D]
    v_wT = np.ascontiguousarray(qkv_w[2 * D:].T)                 # [D, D]
    out_wT = np.ascontiguousarray(out_w.T)                       # [D, D]
    qkb = np.ascontiguousarray(qkv_b[:2 * D].reshape(2 * DC, 128).T)
    out_b_eff = out_b + out_w @ qkv_b[2 * D:]
    outb = np.ascontiguousarray(out_b_eff.reshape(DC, 128).T)
    kk, qq = np.meshgrid(np.arange(128), np.arange(128), indexing="ij")
    mask = (kk <= qq).astype(np.float32)
    ones = np.ones((128, 1), np.float32)
    zeros = np.zeros((128, L), np.float32)

    shared = {"qkw": qk_wT, "vw": v_wT, "ow": out_wT, "qkb": qkb,
              "outb": outb, "mask": mask, "ones": ones, "zeros": zeros}
    return [dict(shared, xt=xT[c]) for c in range(B)]


def kernel(x, qkv_w, qkv_b, out_w, out_b):
    from concourse.bass_utils import run_bass_kernel_spmd

    if "nc" not in _cache:
        _cache["nc"] = _build()
    nc = _cache["nc"]

    in_maps = _prep_inputs(x, qkv_w, qkv_b, out_w, out_b)
    res = run_bass_kernel_spmd(nc, in_maps, core_ids=list(range(B)))
    out = np.empty((B, L, D), np.float32)
    for c in range(B):
        out[c] = res.results[c]["o"].T
    return out


# revision 19
# speedup vs baseline: 2.1402x; 2.0239x over previous
"""Causal self-attention kernel for Trainium2 (8 NeuronCores, data-parallel).

Shapes (hardcoded): x [8, 1024, 640], qkv_w [1920, 640], qkv_b [1920],
out_w [640, 640], out_b [640].  B=8 batch elements -> one per core.

Per-core layout strategy: keep activations feature-major ("transposed") so
every matmul contraction dim sits on SBUF partitions:
  xT [640, 1024]          (host-transposed input)
  qT = q_wT.T @ xT        [640, 1024] chunks (+bias)
  kT -> k_pad_e/k_pad_o   zero-padded to K=128 so QK matmuls use the full
                          PE array (half-array matmuls don't register as
                          activity for the HAM clock gate -> 1.2GHz)
  v   = xT.T @ v_wT       [1024, 640]   (k-position major, ready as AV lhsT)
  attT[k,q] = kT.T @ qT   per head, exp with causal mask, no max-subtract
                          (|logits| <= ~1.4 for this problem's scale)
  yT_aug = v_aug.T @ expT [65+, q]      (row 64 = softmax denominators)
  outT = out_wT.T @ yT    [640, 1024]   (host transposes back)
All matmuls run as float32r (1 cycle/row for N>=256 vs 4 for fp32).
v-projection bias is folded into the output bias on the host (softmax rows
sum to 1, so att @ (1 v_b^T) == 1 v_b^T exactly).
Phase-B leftovers and the first half of the output projection are emitted
as fillers inside the attention loop to keep the PE fed while ACT/DVE work
on exp/normalization.
"""

import numpy as np

B, L, D = 8, 1024, 640
H, DH = 10, 64
DC = D // 128          # 5 contraction chunks
SCALE = DH ** -0.5

_cache = {}


def _build():
    import contextlib

    import concourse.bacc as bacc
    import concourse.mybir as mybir
    import concourse.tile as tile

    f32 = mybir.dt.float32
    f32r = mybir.dt.float32r
    Act = mybir.ActivationFunctionType

    nc = bacc.Bacc("TRN2", target_bir_lowering=False)

    xt_d = nc.dram_tensor("xt", [D, L], f32r, kind="ExternalInput")
    qkw_d = nc.dram_tensor("qkw", [D, 2 * D], f32r, kind="ExternalInput")
    vw_d = nc.dram_tensor("vw", [D, D], f32r, kind="ExternalInput")
    ow_d = nc.dram_tensor("ow", [D, D], f32r, kind="ExternalInput")
    qkb_d = nc.dram_tensor("qkb", [128, 2 * DC], f32, kind="ExternalInput")
    outb_d = nc.dram_tensor("outb", [128, DC], f32, kind="ExternalInput")
    mask_d = nc.dram_tensor("mask", [128, 128], f32r, kind="ExternalInput")
    ones_d = nc.dram_tensor("ones", [128, H], f32r, kind="ExternalInput")
    zeros_d = nc.dram_tensor("zeros", [128, L], f32r, kind="ExternalInput")
    o_d = nc.dram_tensor("o", [D, L], f32, kind="ExternalOutput")

    with tile.TileContext(nc) as tc:
        with contextlib.ExitStack() as ctx:
            singles = ctx.enter_context(tc.tile_pool(name="singles", bufs=1))
            expp = ctx.enter_context(tc.tile_pool(name="expp", bufs=4))
            normp = ctx.enter_context(tc.tile_pool(name="normp", bufs=4))
            outp = ctx.enter_context(tc.tile_pool(name="outp", bufs=3))
            owp = ctx.enter_context(tc.tile_pool(name="owp", bufs=6))
            ps_proj = ctx.enter_context(tc.tile_pool(name="ps_proj", bufs=2, space="PSUM"))
            ps_att = ctx.enter_context(tc.tile_pool(name="ps_att", bufs=3, space="PSUM"))
            ps_y = ctx.enter_context(tc.tile_pool(name="ps_y", bufs=3, space="PSUM"))

            # ---- resident SBUF tensors ----
            xt_s = singles.tile([128, DC, L], f32r)        # xT
            qkw_s = singles.tile([128, DC, 2 * D], f32r)   # [q_wT | k_wT]
            vw_s = singles.tile([128, DC, D], f32r)        # v_wT
            q_s = singles.tile([128, DC, L], f32r)         # qT (+bias)
            k_pad_e = singles.tile([128, DC, L], f32r)     # [kT_even; 0]
            k_pad_o = singles.tile([128, DC, L], f32r)     # [0; kT_odd]
            v_s = singles.tile([128, L // 128, H * 65 + 63], f32r)
            yt_s = singles.tile([128, DC, L], f32r)        # normalized yT
            qkb_s = singles.tile([128, 2 * DC], f32)
            outb_s = singles.tile([128, DC], f32)
            mask_s = singles.tile([128, 128], f32r)
            zeros_s = singles.tile([128, L], f32r)
            ones_s = singles.tile([128, H], f32r)

            # ---- input DMAs (emission order ~= queue order) ----
            for dc in range(DC):
                nc.sync.dma_start(out=xt_s[:, dc, :], in_=xt_d[128 * dc:128 * dc + 128, :])
            nc.sync.dma_start(out=qkb_s, in_=qkb_d[:, :])
            # q-half of qkw first (unblocks B), then vw (unblocks C), then k-half
            for dc in range(DC):
                nc.sync.dma_start(out=qkw_s[:, dc, 0:D],
                                  in_=qkw_d[128 * dc:128 * dc + 128, 0:D])
            for dc in range(DC):
                nc.sync.dma_start(out=vw_s[:, dc, :], in_=vw_d[128 * dc:128 * dc + 128, :])
            for dc in range(DC):
                nc.sync.dma_start(out=qkw_s[:, dc, D:2 * D],
                                  in_=qkw_d[128 * dc:128 * dc + 128, D:2 * D])
            nc.sync.dma_start(out=mask_s, in_=mask_d[:, :])
            nc.sync.dma_start(out=outb_s, in_=outb_d[:, :])
            nc.sync.dma_start(out=zeros_s, in_=zeros_d[:, :])
            nc.sync.dma_start(out=ones_s, in_=ones_d[:, :])

            # ---- phase B: one (mi, nn) PSUM group = 5 matmuls + ACT ----
            def emit_b_group(mi, nn):
                pt = ps_proj.tile([128, 512], f32, tag="proj", name="ptl")
                for dc in range(DC):
                    nc.tensor.matmul(
                        pt[:, :],
                        qkw_s[:, dc, 128 * mi:128 * mi + 128],
                        xt_s[:, dc, 512 * nn:512 * nn + 512],
                        start=(dc == 0), stop=(dc == DC - 1))
                sl = slice(512 * nn, 512 * nn + 512)
                if mi < DC:            # q chunk
                    nc.scalar.activation(q_s[:, mi, sl], pt[:, :],
                                         Act.Identity, bias=qkb_s[:, mi:mi + 1])
                else:                  # k chunk -> zero-padded halves
                    hp = mi - DC
                    nc.scalar.activation(k_pad_e[0:64, hp, sl], pt[0:64, :],
                                         Act.Identity, bias=qkb_s[0:64, mi:mi + 1])
                    nc.scalar.activation(k_pad_o[64:128, hp, sl], pt[64:128, :],
                                         Act.Identity, bias=qkb_s[64:128, mi:mi + 1])
                    nc.vector.tensor_copy(out=k_pad_e[64:128, hp, sl],
                                          in_=zeros_s[64:128, sl])
                    nc.vector.tensor_copy(out=k_pad_o[0:64, hp, sl],
                                          in_=zeros_s[0:64, sl])

            def emit_b_chunk(mi):
                for nn in range(2):
                    emit_b_group(mi, nn)

            # ---- phase E: one jc group (ow streamed from DRAM) ----
            def emit_e_chunk(jc):
                ow_tiles = []
                for dc in range(DC):
                    t = owp.tile([128, 128], f32r, tag="ow", name="owt")
                    nc.sync.dma_start(
                        out=t,
                        in_=ow_d[128 * dc:128 * dc + 128, 128 * jc:128 * jc + 128])
                    ow_tiles.append(t)
                for nn in range(2):
                    pt = ps_proj.tile([128, 512], f32, tag="proj", name="ptl")
                    for dc in range(DC):
                        nc.tensor.matmul(
                            pt[:, :],
                            ow_tiles[dc][:, :],
                            yt_s[:, dc, 512 * nn:512 * nn + 512],
                            start=(dc == 0), stop=(dc == DC - 1))
                    ot = outp.tile([128, 512], f32, tag="ot")
                    nc.scalar.activation(ot[:, :], pt[:, :],
                                         Act.Identity, bias=outb_s[:, jc:jc + 1])
                    nc.sync.dma_start(
                        out=o_d[128 * jc:128 * jc + 128, 512 * nn:512 * nn + 512],
                        in_=ot[:, :])

            # ---- phase C: v = xT.T @ v_wT ----
            def emit_c():
                for li in range(L // 128):
                    pt = ps_proj.tile([128, 512], f32, tag="proj")
                    for dc in range(DC):
                        nc.tensor.matmul(
                            pt[:, :],
                            xt_s[:, dc, 128 * li:128 * li + 128],
                            vw_s[:, dc, 0:512],
                            start=(dc == 0), stop=(dc == DC - 1))
                    dst = v_s[:, li, 0:520].rearrange("p (h c) -> p h c", c=65)[:, :, 0:64]
                    nc.vector.tensor_copy(out=dst, in_=pt[:, :].rearrange("p (h c) -> p h c", c=64))
                    pt2 = ps_proj.tile([128, 128], f32, tag="proj")
                    for dc in range(DC):
                        nc.tensor.matmul(
                            pt2[:, :],
                            xt_s[:, dc, 128 * li:128 * li + 128],
                            vw_s[:, dc, 512:640],
                            start=(dc == 0), stop=(dc == DC - 1))
                    dst2 = v_s[:, li, 520:650].rearrange("p (h c) -> p h c", c=65)[:, :, 0:64]
                    nc.vector.tensor_copy(out=dst2, in_=pt2[:, :].rearrange("p (h c) -> p h c", c=64))
                    ocols = v_s[:, li, 0:650].rearrange("p (h c) -> p h c", c=65)[:, :, 64]
                    nc.vector.tensor_copy(out=ocols, in_=ones_s[:, :])
                    nc.vector.tensor_copy(out=v_s[:, li, H * 65:], in_=zeros_s[:, 0:63])

            # ---- phase D: attention ----
            def attend(h, qc):
                kp = k_pad_o if (h % 2) else k_pad_e
                po = 64 * (h % 2)
                qT2 = q_s[:, h // 2, :]
                py = ps_y.tile([128, 512], f32, tag="py")
                nki = 4 * qc + 4
                for ki in range(nki):
                    off = 128 * ki - 512 * qc
                    lo = max(0, off)
                    pa = ps_att.tile([128, 512], f32, tag="pa")
                    nc.tensor.matmul(
                        pa[:, lo:512],
                        kp[:, h // 2, 128 * ki:128 * ki + 128],
                        qT2[:, 512 * qc + lo:512 * qc + 512],
                        start=True, stop=True)
                    ex = expp.tile([128, 512], f32r, tag="ex")
                    nc.scalar.activation(ex[:, lo:512], pa[:, lo:512], Act.Exp, scale=SCALE)
                    if off >= 0:
                        nc.vector.tensor_mul(
                            ex[:, off:off + 128], ex[:, off:off + 128], mask_s[:, :])
                    nc.tensor.matmul(
                        py[:, lo:512],
                        v_s[:, ki, 65 * h:65 * h + 128],
                        ex[:, lo:512],
                        start=(ki == 0), stop=(ki == nki - 1))
                den = normp.tile([1, 512], f32, tag="den")
                nc.vector.tensor_copy(out=den[:, :], in_=py[64:65, :])
                recip = normp.tile([1, 512], f32, tag="rc")
                nc.vector.reciprocal_approx_fast(out=recip[:, :], in_=den[:, :])
                rb = normp.tile([64, 512], f32, tag="rb")
                nc.gpsimd.partition_broadcast(rb[:, :], recip[:, :])
                nc.vector.tensor_mul(
                    yt_s[po:po + 64, h // 2, 512 * qc:512 * qc + 512],
                    py[0:64, :], rb[:, :])

            # ---- emission schedule ----
            for mi in range(DC):
                emit_b_chunk(mi)          # q chunks
            for mi in range(DC, 2 * DC):
                emit_b_chunk(mi)          # k chunks -> k_pad
            emit_c()
            for p in range(DC):
                for qc in range(2):
                    attend(2 * p, qc)
                    attend(2 * p + 1, qc)
            for jc in range(DC):
                emit_e_chunk(jc)

    nc.compile()
    return nc


def _prep_inputs(x, qkv_w, qkv_b, out_w, out_b):
    x = np.ascontiguousarray(x, dtype=np.float32)
    qkv_w = np.asarray(qkv_w, dtype=np.float32)
    qkv_b = np.asarray(qkv_b, dtype=np.float32)
    out_w = np.asarray(out_w, dtype=np.float32)
    out_b = np.asarray(out_b, dtype=np.float32)

    xT = np.ascontiguousarray(x.transpose(0, 2, 1))              # [B, D, L]
    qk_wT = np.ascontiguousarray(qkv_w[:2 * D].T)                # [D, 2D]
    v_wT = np.ascontiguousarray(qkv_w[2 * D:].T)                 # [D, D]
    out_wT = np.ascontiguousarray(out_w.T)                       # [D, D]
    qkb = np.ascontiguousarray(qkv_b[:2 * D].reshape(2 * DC, 128).T)
    out_b_eff = out_b + out_w @ qkv_b[2 * D:]
    outb = np.ascontiguousarray(out_b_eff.reshape(DC, 128).T)
    kk, qq = np.meshgrid(np.arange(128), np.arange(128), indexing="ij")
    mask = (kk <= qq).astype(np.float32)
    ones = np.ones((128, H), np.float32)
    zeros = np.zeros((128, L), np.float32)

    shared = {"qkw": qk_wT, "vw": v_wT, "ow": out_wT, "qkb": qkb,
              "outb": outb, "mask": mask, "ones": ones, "zeros": zeros}
    return [dict(shared, xt=xT[c]) for c in range(B)]


def kernel(x, qkv_w, qkv_b, out_w, out_b):
    from concourse.bass_utils import run_bass_kernel_spmd

    if "nc" not in _cache:
        _cache["nc"] = _build()
    nc = _cache["nc"]

    in_maps = _prep_inputs(x, qkv_w, qkv_b, out_w, out_b)
    res = run_bass_kernel_spmd(nc, in_maps, core_ids=list(range(B)))
    out = np.empty((B, L, D), np.float32)
    for c in range(B):
        out[c] = res.results[c]["o"].T
    return out
